# revision 23
# baseline (speedup 1.0000x reference)
"""Trainium2 Bass kernel for the LstmPredictor module (v2).

Model (per batch element b):
    h   = relu(x @ w_in_k + w_in_b)            # (T=20, 64)
    enc = LSTM_256(h)[-1]                      # (256,)
    dec = LSTM_256(repeat(enc, 15))            # (15, 256)  (return_seq)
    out = [dec @ mean_k + mean_b, relu(dec @ lv_k + lv_b)]   # (15, 4)

Strategy: pure data parallel over batch (8192 -> 8 cores x 1024).
Feature-major SBUF layout (batch in the free dim); all matmul operands in
bf16 (fast weight load + halved DMA), PSUM/c-state in fp32.

Per core the batch is split into 2 independent 512-wide chunk pipelines,
each owning 4 PSUM banks used in two waves per LSTM step:
  wave1 {i,f}: 12 matmuls -> one sigmoid drain (N=2048)
  wave2 {g,o}: 12 matmuls -> tanh(g) + sigmoid(o) drains
DVE does the cell update (ig/fc/add/h; bf16 2x where possible), ACT does
only sigmoid/tanh. Biases ride in the xh ones-row (encoder), inside zdx
(decoder), and in the DVE head drain (head), so no extra bias matmuls in
the scan. The decoder head (mean/log_var) runs in-scan into a 1-bank PSUM
tile, drained by DVE tensor_scalar (bias add + relu fused) straight to
DMA. P1 (input projection) drains via DVE tensor_scalar (bias+relu),
interleaved with the first encoder steps.
"""

import numpy as np
import ml_dtypes

import concourse.bass as bass
import concourse.mybir as mybir
import concourse.tile as tile
from concourse import bacc, bass_utils
from concourse.bass import ds, ts

N_CORES = 8
B_FULL = 8192
BC = B_FULL // N_CORES  # 1024 batch per core
NCH = 2  # chunks per core
CW = BC // NCH  # 512
T_ENC = 20
T_DEC = 15
H = 256
BF = mybir.dt.bfloat16
F32 = mybir.dt.float32
AF = mybir.ActivationFunctionType
ALU = mybir.AluOpType

LAST_RESULTS = None  # BassKernelResults of the most recent run (for test.py)
_NC_CACHE = []


def _build_nc():
    nc = bacc.Bacc("TRN2", target_bir_lowering=False, debug=False, num_devices=N_CORES)

    # ---- DRAM I/O (per-core shapes; host marshals layouts) ----
    xt_d = nc.dram_tensor("xt", [128, 7, NCH, CW], BF, kind="ExternalInput")
    wink_d = nc.dram_tensor("w_in_k", [128, 64], BF, kind="ExternalInput")
    winb_d = nc.dram_tensor("w_in_b64", [64, 1], F32, kind="ExternalInput")
    enck_d = nc.dram_tensor("enc_k_ext", [65, 4 * H], BF, kind="ExternalInput")
    encrk_d = nc.dram_tensor("enc_rk", [128, 2, 4 * H], BF, kind="ExternalInput")
    deck_d = nc.dram_tensor("dec_k", [128, 2, 4 * H], BF, kind="ExternalInput")
    deckb_d = nc.dram_tensor("dec_b", [1, 4 * H], BF, kind="ExternalInput")
    decrk_d = nc.dram_tensor("dec_rk", [128, 2, 4 * H], BF, kind="ExternalInput")
    whead_d = nc.dram_tensor("w_head", [128, 2, 4], BF, kind="ExternalInput")
    hbias_d = nc.dram_tensor("head_bias", [4, 1], F32, kind="ExternalInput")
    hgate_d = nc.dram_tensor("head_gate", [4, 1], F32, kind="ExternalInput")
    ident_d = nc.dram_tensor("ident", [128, 128], BF, kind="ExternalInput")
    ones_d = nc.dram_tensor("ones", [1, T_ENC * BC], BF, kind="ExternalInput")

    # out rows: (t, [mean0 mean1 lv0 lv1]) x (chunk, n) columns
    oh_d = nc.dram_tensor("out_head", [T_DEC, 4, NCH, CW], F32, kind="ExternalOutput")

    with tile.TileContext(nc) as tc:
        with tc.tile_pool(name="stat", bufs=1) as stat:
            # ---- persistent SBUF tensors ----
            wink = stat.tile([128, 64], BF, tag="wink")
            winb = stat.tile([64, 1], F32, tag="winb")
            enck = stat.tile([65, 4 * H], BF, tag="enck")
            encrk = stat.tile([128, 2, 4 * H], BF, tag="encrk")
            deck = stat.tile([128, 2, 4 * H], BF, tag="deck")
            deckb = stat.tile([1, 4 * H], BF, tag="deckb")
            decrk = stat.tile([128, 2, 4 * H], BF, tag="decrk")
            whead = stat.tile([128, 2, 4], BF, tag="whead")
            hbias = stat.tile([4, 1], F32, tag="hbias")
            hgate = stat.tile([4, 1], F32, tag="hgate")
            ident = stat.tile([128, 128], BF, tag="ident")
            ones = stat.tile([1, CW], BF, tag="ones")
            # per-chunk recurrent state (hT rotates through gsb tiles)
            hT = [None, None]
            cT = [stat.tile([128, 2, CW], F32, tag=f"cT{c}", name=f"cT{c}") for c in range(NCH)]
            fcs = [stat.tile([128, 2, CW], F32, tag=f"fcs{c}", name=f"fcs{c}") for c in range(NCH)]

            nc.sync.dma_start(out=wink, in_=wink_d[:, :])
            nc.sync.dma_start(out=winb, in_=winb_d[:, :])

            with (
                tc.tile_pool(name="psc0", bufs=1, space="PSUM") as psc0,
                tc.tile_pool(name="psc1", bufs=1, space="PSUM") as psc1,
                tc.tile_pool(name="gsb", bufs=2) as gsb,
                tc.tile_pool(name="encsb", bufs=1) as encsb,
            ):
                pschunk = [psc0, psc1]
                # xh: input projection, transposed, with trailing ones row
                # for the encoder bias trick (enck row 64 = enc_b)
                xt = encsb.tile([128, 7, NCH, CW], BF, tag="xt")
                xh = encsb.tile([65, T_ENC, NCH, CW], BF, tag="xh")
                nc.sync.dma_start(out=xt[:, 0:1, :, :], in_=xt_d[:, 0:1, :, :])
                nc.sync.dma_start(out=ones, in_=ones_d[0:1, 0:CW])
                nc.sync.dma_start(out=enck, in_=enck_d[:, :])
                nc.sync.dma_start(out=xh[64:65, :, :, :], in_=ones_d.ap())
                nc.sync.dma_start(out=encrk, in_=encrk_d.ap())
                nc.sync.dma_start(out=xt[:, 1:3, :, :], in_=xt_d[:, 1:3, :, :])
                nc.sync.dma_start(out=xt[:, 3:7, :, :], in_=xt_d[:, 3:7, :, :])
                nc.sync.dma_start(out=deck, in_=deck_d.ap())
                nc.sync.dma_start(out=deckb, in_=deckb_d[:, :])
                nc.sync.dma_start(out=decrk, in_=decrk_d.ap())
                nc.sync.dma_start(out=whead, in_=whead_d.ap())
                nc.sync.dma_start(out=hbias, in_=hbias_d[:, :])
                nc.sync.dma_start(out=hgate, in_=hgate_d[:, :])
                nc.sync.dma_start(out=ident, in_=ident_d[:, :])

                # ---- P1: xh[t] = relu(x_t @ w_in_k + b); 2-timestep packs
                # per chunk, borrowing that chunk's slot-A PSUM banks ----
                def p1_pack(c, t0):
                    pin = pschunk[c].tile([64, 2, CW], F32, tag=f"psA{c}", name="pin")
                    for j in range(2):
                        t = t0 + j
                        bp = (t % 3) * 32
                        nc.tensor.matmul(
                            pin[:, j, :],
                            wink[ds(bp, 8), :],
                            xt[ds(bp, 8), t // 3, c, :],
                            start=True, stop=True,
                        )
                    # bias + relu fused in one DVE tensor_scalar, bf16 out
                    nc.vector.tensor_scalar(
                        out=xh[0:64, t0 : t0 + 2, c, :],
                        in0=pin,
                        scalar1=winb[:, 0:1],
                        scalar2=0.0,
                        op0=ALU.add,
                        op1=ALU.max,
                    )

                def mm_gate(c, t, g, xmm, rk, name):
                    """6 matmuls for one gate (2 m-tiles) into a 2-bank tile.
                    Gates g: 0=i (slot A), 1=f (slot B), 2=g (slot A),
                    3=o (slot B)."""
                    slot = "A" if g % 2 == 0 else "B"
                    pt2 = pschunk[c].tile(
                        [128, 2, CW], F32, tag=f"ps{slot}{c}", name=name
                    )
                    for mi in range(2):
                        m = g * 2 + mi
                        pt = pt2[:, mi, :]
                        xmm(pt, m)
                        if t > 0:
                            nc.tensor.matmul(
                                pt, rk[:, 0, ts(m, 128)], hT[c][:, 0, :],
                                start=False, stop=False,
                            )
                            nc.tensor.matmul(
                                pt, rk[:, 1, ts(m, 128)], hT[c][:, 1, :],
                                start=False, stop=True,
                            )
                    return pt2

                # deferred head state: (t, chunk) whose th-psum awaits drain
                def head_mm(c, t):
                    th = pschunk[c].tile([4, CW], F32, tag=f"psB{c}", name="th")
                    nc.tensor.matmul(
                        th[:, :], whead[:, 0, :], hT[c][:, 0, :],
                        start=True, stop=False,
                    )
                    nc.tensor.matmul(
                        th[:, :], whead[:, 1, :], hT[c][:, 1, :],
                        start=False, stop=True,
                    )
                    return th

                def head_drain(c, t, th):
                    stg = gsb.tile([4, CW], F32, tag=f"stg{c}", name="stg")
                    # +bias on all rows; relu only on lv rows via per-partition
                    # max operand (-inf on mean rows)
                    nc.vector.tensor_scalar(
                        out=stg, in0=th,
                        scalar1=hbias[:, 0:1], scalar2=hgate[:, 0:1],
                        op0=ALU.add, op1=ALU.max,
                    )
                    nc.sync.dma_start(out=oh_d[t, :, c, :], in_=stg)

                def chunk_step(c, t, xmm, rk, head_t=None):
                    """One full LSTM step for chunk c; PE flows i,f,g,o with
                    drains interleaved so no engine head-blocks >~1.2us."""
                    first = t == 0
                    if head_t is not None and head_t > 0:
                        # head for the PREVIOUS step of this chunk (h ready)
                        th = head_mm(c, head_t - 1)
                        head_drain(c, head_t - 1, th)
                    ti = mm_gate(c, t, 0, xmm, rk, "ti")
                    tf = mm_gate(c, t, 1, xmm, rk, "tf")
                    si = gsb.tile([128, 2, CW], BF, tag=f"si{c}", name="si")
                    nc.scalar.activation(out=si, in_=ti, func=AF.Sigmoid)
                    tgp = mm_gate(c, t, 2, xmm, rk, "tgp")
                    sf = gsb.tile([128, 2, CW], BF, tag=f"sf{c}", name="sf")
                    nc.scalar.activation(out=sf, in_=tf, func=AF.Sigmoid)
                    top = mm_gate(c, t, 3, xmm, rk, "top")
                    tg = gsb.tile([128, 2, CW], BF, tag=f"tg{c}", name="tg")
                    nc.scalar.activation(out=tg, in_=tgp, func=AF.Tanh)
                    if not first:
                        # fc on GPSIMD (off the DVE critical path)
                        nc.gpsimd.tensor_mul(fcs[c], sf, cT[c])
                    so = gsb.tile([128, 2, CW], BF, tag=f"so{c}", name="so")
                    nc.scalar.activation(out=so, in_=top, func=AF.Sigmoid)
                    tc_t = gsb.tile([128, 2, CW], BF, tag=f"tc{c}", name="tc")
                    if first:
                        nc.vector.tensor_mul(cT[c], si, tg)
                    else:
                        ig = gsb.tile([128, 2, CW], BF, tag=f"ig{c}", name="ig")
                        nc.vector.tensor_mul(ig, si, tg)  # bf16 2x
                        nc.vector.tensor_add(cT[c], fcs[c], ig)
                    nc.scalar.activation(out=tc_t, in_=cT[c], func=AF.Tanh)
                    hT[c] = gsb.tile([128, 2, CW], BF, tag=f"hT{c}", name="hT")
                    nc.vector.tensor_mul(hT[c], so, tc_t)  # bf16 2x
                    if head_t is not None and head_t == T_DEC - 1:
                        th = head_mm(c, head_t)
                        head_drain(c, head_t, th)

                # ---- P2: encoder (P1 packs interleaved ahead of use) ----
                for c in range(NCH):
                    p1_pack(c, 0)
                for t in range(T_ENC):
                    for c in range(NCH):
                        if 1 <= t <= 9:
                            p1_pack(c, 2 * t)

                        def enc_xmm(pt, m, t=t, c=c):
                            nc.tensor.matmul(
                                pt, enck[:, ts(m, 128)], xh[:, t, c, :],
                                start=True, stop=False if t > 0 else True,
                            )

                        chunk_step(c, t, enc_xmm, encrk)

                # ---- P3: zdx = dec_k^T enc_h + dec_b; decoder t=0 gates
                # are fused straight from the zdx PSUM tiles (no inject) ----
                zdx = encsb.tile([128, 8, NCH, CW], BF, tag="zdx")
                GFN = [AF.Sigmoid, AF.Sigmoid, AF.Tanh, AF.Sigmoid]
                GTAG = ["si", "sf", "tg", "so"]
                t0g = {}
                for g in range(4):
                    pz = {}
                    for c in range(NCH):
                        slot = "A" if g % 2 == 0 else "B"
                        pz[c] = pschunk[c].tile(
                            [128, 2, CW], F32, tag=f"ps{slot}{c}", name="pz"
                        )
                        for mi in range(2):
                            m = g * 2 + mi
                            nc.tensor.matmul(
                                pz[c][:, mi, :], deck[:, 0, ts(m, 128)], hT[c][:, 0, :],
                                start=True, stop=False,
                            )
                            nc.tensor.matmul(
                                pz[c][:, mi, :], deck[:, 1, ts(m, 128)], hT[c][:, 1, :],
                                start=False, stop=False,
                            )
                            nc.tensor.matmul(
                                pz[c][:, mi, :], deckb[:, ts(m, 128)], ones[:, :],
                                start=False, stop=True,
                            )
                    for c in range(NCH):
                        gt = gsb.tile([128, 2, CW], BF, tag=f"{GTAG[g]}{c}", name="t0g")
                        nc.scalar.activation(out=gt, in_=pz[c], func=GFN[g])
                        t0g[(g, c)] = gt
                        if g <= 1:
                            nc.vector.tensor_copy(zdx[:, ds(g * 2, 2), c, :], pz[c])
                        else:
                            nc.scalar.activation(
                                out=zdx[:, ds(g * 2, 2), c, :], in_=pz[c], func=AF.Copy
                            )
                # decoder t=0 cell update (c0 = i*g, h0 = o*tanh(c0))
                for c in range(NCH):
                    nc.vector.tensor_mul(cT[c], t0g[(0, c)], t0g[(2, c)])
                    tc0 = gsb.tile([128, 2, CW], BF, tag=f"tc{c}", name="tc0")
                    nc.scalar.activation(out=tc0, in_=cT[c], func=AF.Tanh)
                    hT[c] = gsb.tile([128, 2, CW], BF, tag=f"hT{c}", name="hT0")
                    nc.vector.tensor_mul(hT[c], t0g[(3, c)], tc0)

                # ---- P4: decoder t>=1 (in-scan head, deferred one step) ----
                for t in range(1, T_DEC):
                    for c in range(NCH):

                        def dec_xmm(pt, m, c=c):
                            nc.tensor.matmul(
                                pt, ident[:, :], zdx[:, m, c, :],
                                start=True, stop=False,
                            )

                        chunk_step(c, t, dec_xmm, decrk, head_t=t)

    nc.compile()
    return nc


def _marshal(x, w_in_k, w_in_b, enc_k, enc_rk, enc_b,
             dec_k, dec_rk, dec_b, mean_k, mean_b, lv_k, lv_b):
    f = np.float32
    bf = ml_dtypes.bfloat16
    x = np.asarray(x, f)
    enck_ext = np.concatenate([np.asarray(enc_k, f), np.asarray(enc_b, f)[None, :]], 0)
    shared = {
        "w_in_k": np.ascontiguousarray(
            np.tile(
                np.pad(np.asarray(w_in_k, bf), ((0, 24), (0, 0))), (4, 1)
            )
        ),
        "w_in_b64": np.ascontiguousarray(np.asarray(w_in_b, f)[:, None]),
        "enc_k_ext": np.ascontiguousarray(enck_ext.astype(bf)),
        "enc_rk": np.ascontiguousarray(
            np.asarray(enc_rk, bf).reshape(2, 128, 4 * H).transpose(1, 0, 2)
        ),
        "dec_k": np.ascontiguousarray(
            np.asarray(dec_k, bf).reshape(2, 128, 4 * H).transpose(1, 0, 2)
        ),
        "dec_b": np.ascontiguousarray(np.asarray(dec_b, bf)[None, :]),
        "dec_rk": np.ascontiguousarray(
            np.asarray(dec_rk, bf).reshape(2, 128, 4 * H).transpose(1, 0, 2)
        ),
        "w_head": np.ascontiguousarray(
            np.concatenate([np.asarray(mean_k, f), np.asarray(lv_k, f)], 1)
            .astype(bf)
            .reshape(2, 128, 4)
            .transpose(1, 0, 2)
        ),
        "head_bias": np.ascontiguousarray(
            np.concatenate([np.asarray(mean_b, f), np.asarray(lv_b, f)])[:, None]
        ),
        "head_gate": np.array([[-3e38], [-3e38], [0.0], [0.0]], f),
        "ident": np.eye(128, dtype=bf),
        "ones": np.ones((1, T_ENC * BC), bf),
    }
    in_maps = []
    for c in range(N_CORES):
        xs = x[c * BC : (c + 1) * BC]  # (BC, 20, 8)
        m = dict(shared)
        arr = xs.transpose(2, 1, 0).astype(bf).reshape(8, T_ENC, NCH, CW)
        xtw = np.zeros((128, 7, NCH, CW), bf)
        for t in range(T_ENC):
            xtw[(t % 3) * 32 : (t % 3) * 32 + 8, t // 3] = arr[:, t]
        m["xt"] = xtw
        in_maps.append(m)
    return in_maps


def _assemble(results):
    outs = []
    for c in range(N_CORES):
        oh = results[c]["out_head"].reshape(T_DEC, 4, BC)  # (t, r, b)
        outs.append(oh.transpose(2, 0, 1))  # (BC, 15, 4)
    return np.ascontiguousarray(np.concatenate(outs, 0))


def _run(trace=False, **inputs):
    global LAST_RESULTS
    if not _NC_CACHE:
        _NC_CACHE.append(_build_nc())
    nc = _NC_CACHE[0]
    in_maps = _marshal(**inputs)
    LAST_RESULTS = bass_utils.run_bass_kernel_spmd(
        nc, in_maps, core_ids=list(range(N_CORES)), trace=trace
    )
    return _assemble(LAST_RESULTS.results)


def kernel(**inputs):
    return _run(trace=False, **inputs)


# revision 24
# speedup vs baseline: 1.0247x; 1.0247x over previous
"""Trainium2 Bass kernel for the LstmPredictor module (v2).

Model (per batch element b):
    h   = relu(x @ w_in_k + w_in_b)            # (T=20, 64)
    enc = LSTM_256(h)[-1]                      # (256,)
    dec = LSTM_256(repeat(enc, 15))            # (15, 256)  (return_seq)
    out = [dec @ mean_k + mean_b, relu(dec @ lv_k + lv_b)]   # (15, 4)

Strategy: pure data parallel over batch (8192 -> 8 cores x 1024).
Feature-major SBUF layout (batch in the free dim); all matmul operands in
bf16 (fast weight load + halved DMA), PSUM/c-state in fp32.

Per core the batch is split into 2 independent 512-wide chunk pipelines,
each owning 4 PSUM banks used in two waves per LSTM step:
  wave1 {i,f}: 12 matmuls -> one sigmoid drain (N=2048)
  wave2 {g,o}: 12 matmuls -> tanh(g) + sigmoid(o) drains
DVE does the cell update (ig/fc/add/h; bf16 2x where possible), ACT does
only sigmoid/tanh. Biases ride in the xh ones-row (encoder), inside zdx
(decoder), and in the DVE head drain (head), so no extra bias matmuls in
the scan. The decoder head (mean/log_var) runs in-scan into a 1-bank PSUM
tile, drained by DVE tensor_scalar (bias add + relu fused) straight to
DMA. P1 (input projection) drains via DVE tensor_scalar (bias+relu),
interleaved with the first encoder steps.
"""

import numpy as np
import ml_dtypes

import concourse.bass as bass
import concourse.mybir as mybir
import concourse.tile as tile
from concourse import bacc, bass_utils
from concourse.bass import ds, ts

N_CORES = 8
B_FULL = 8192
BC = B_FULL // N_CORES  # 1024 batch per core
NCH = 2  # chunks per core
CW = BC // NCH  # 512
T_ENC = 20
T_DEC = 15
H = 256
BF = mybir.dt.bfloat16
F32 = mybir.dt.float32
AF = mybir.ActivationFunctionType
ALU = mybir.AluOpType

LAST_RESULTS = None  # BassKernelResults of the most recent run (for test.py)
_NC_CACHE = []


def _build_nc():
    nc = bacc.Bacc("TRN2", target_bir_lowering=False, debug=False, num_devices=N_CORES)

    # ---- DRAM I/O (per-core shapes; host marshals layouts) ----
    xt_d = nc.dram_tensor("xt", [128, 7, NCH, CW], BF, kind="ExternalInput")
    wink_d = nc.dram_tensor("w_in_k", [128, 64], BF, kind="ExternalInput")
    winb_d = nc.dram_tensor("w_in_b64", [64, 1], F32, kind="ExternalInput")
    enck_d = nc.dram_tensor("enc_k_ext", [65, 4 * H], BF, kind="ExternalInput")
    encrk_d = nc.dram_tensor("enc_rk", [128, 2, 4 * H], BF, kind="ExternalInput")
    deck_d = nc.dram_tensor("dec_k", [128, 2, 4 * H], BF, kind="ExternalInput")
    deckb_d = nc.dram_tensor("dec_b", [1, 4 * H], BF, kind="ExternalInput")
    decrk_d = nc.dram_tensor("dec_rk", [128, 2, 4 * H], BF, kind="ExternalInput")
    whead_d = nc.dram_tensor("w_head", [128, 2, 4], BF, kind="ExternalInput")
    hbias_d = nc.dram_tensor("head_bias", [4, 1], F32, kind="ExternalInput")
    hgate_d = nc.dram_tensor("head_gate", [4, 1], F32, kind="ExternalInput")
    ident_d = nc.dram_tensor("ident", [128, 128], BF, kind="ExternalInput")
    ones_d = nc.dram_tensor("ones", [1, T_ENC * BC], BF, kind="ExternalInput")

    # out rows: (t, [mean0 mean1 lv0 lv1]) x (chunk, n) columns
    oh_d = nc.dram_tensor("out_head", [T_DEC, 4, NCH, CW], F32, kind="ExternalOutput")

    with tile.TileContext(nc) as tc:
        with tc.tile_pool(name="stat", bufs=1) as stat:
            # ---- persistent SBUF tensors ----
            wink = stat.tile([128, 64], BF, tag="wink")
            winb = stat.tile([64, 1], F32, tag="winb")
            enck = stat.tile([65, 4 * H], BF, tag="enck")
            encrk = stat.tile([128, 2, 4 * H], BF, tag="encrk")
            deck = stat.tile([128, 2, 4 * H], BF, tag="deck")
            deckb = stat.tile([1, 4 * H], BF, tag="deckb")
            decrk = stat.tile([128, 2, 4 * H], BF, tag="decrk")
            whead = stat.tile([128, 2, 4], BF, tag="whead")
            hbias = stat.tile([4, 1], F32, tag="hbias")
            hgate = stat.tile([4, 1], F32, tag="hgate")
            ident = stat.tile([128, 128], BF, tag="ident")
            ones = stat.tile([1, CW], BF, tag="ones")
            # per-chunk recurrent state (hT rotates through gsb tiles)
            hT = [None, None]
            cT = [stat.tile([128, 2, CW], F32, tag=f"cT{c}", name=f"cT{c}") for c in range(NCH)]
            fcs = [stat.tile([128, 2, CW], F32, tag=f"fcs{c}", name=f"fcs{c}") for c in range(NCH)]

            nc.sync.dma_start(out=wink, in_=wink_d[:, :])
            nc.sync.dma_start(out=winb, in_=winb_d[:, :])

            with (
                tc.tile_pool(name="psc0", bufs=1, space="PSUM") as psc0,
                tc.tile_pool(name="psc1", bufs=1, space="PSUM") as psc1,
                tc.tile_pool(name="gsb", bufs=2) as gsb,
                tc.tile_pool(name="encsb", bufs=1) as encsb,
            ):
                pschunk = [psc0, psc1]
                # xh: input projection, transposed, with trailing ones row
                # for the encoder bias trick (enck row 64 = enc_b)
                xt = encsb.tile([128, 7, NCH, CW], BF, tag="xt")
                xh = encsb.tile([65, T_ENC, NCH, CW], BF, tag="xh")
                nc.sync.dma_start(out=xt[:, 0:1, :, :], in_=xt_d[:, 0:1, :, :])
                nc.sync.dma_start(out=ones, in_=ones_d[0:1, 0:CW])
                nc.sync.dma_start(out=enck, in_=enck_d[:, :])
                nc.sync.dma_start(out=xh[64:65, :, :, :], in_=ones_d.ap())
                nc.sync.dma_start(out=encrk, in_=encrk_d.ap())
                nc.sync.dma_start(out=xt[:, 1:3, :, :], in_=xt_d[:, 1:3, :, :])
                nc.sync.dma_start(out=xt[:, 3:7, :, :], in_=xt_d[:, 3:7, :, :])
                nc.sync.dma_start(out=deck, in_=deck_d.ap())
                nc.sync.dma_start(out=deckb, in_=deckb_d[:, :])
                nc.sync.dma_start(out=decrk, in_=decrk_d.ap())
                nc.sync.dma_start(out=whead, in_=whead_d.ap())
                nc.sync.dma_start(out=hbias, in_=hbias_d[:, :])
                nc.sync.dma_start(out=hgate, in_=hgate_d[:, :])
                nc.sync.dma_start(out=ident, in_=ident_d[:, :])

                # ---- P1: xh[t] = relu(x_t @ w_in_k + b); 2-timestep packs
                # per chunk, borrowing that chunk's slot-A PSUM banks ----
                def p1_pack(c, t0):
                    pin = pschunk[c].tile([64, 2, CW], F32, tag=f"psA{c}", name="pin")
                    for j in range(2):
                        t = t0 + j
                        bp = (t % 3) * 32
                        nc.tensor.matmul(
                            pin[:, j, :],
                            wink[ds(bp, 8), :],
                            xt[ds(bp, 8), t // 3, c, :],
                            start=True, stop=True,
                        )
                    # bias + relu fused in one DVE tensor_scalar, bf16 out
                    nc.vector.tensor_scalar(
                        out=xh[0:64, t0 : t0 + 2, c, :],
                        in0=pin,
                        scalar1=winb[:, 0:1],
                        scalar2=0.0,
                        op0=ALU.add,
                        op1=ALU.max,
                    )

                def mm_gate(c, t, g, xmm, rk, name):
                    """6 matmuls for one gate (2 m-tiles) into a 2-bank tile.
                    Gates g: 0=i (slot A), 1=f (slot B), 2=g (slot A),
                    3=o (slot B)."""
                    slot = "A" if g % 2 == 0 else "B"
                    pt2 = pschunk[c].tile(
                        [128, 2, CW], F32, tag=f"ps{slot}{c}", name=name
                    )
                    # inject passes first (consecutive identical stationary
                    # for the decoder's ident), then the recurrent passes
                    for mi in range(2):
                        xmm(pt2[:, mi, :], g * 2 + mi)
                    if t > 0:
                        for mi in range(2):
                            m = g * 2 + mi
                            pt = pt2[:, mi, :]
                            nc.tensor.matmul(
                                pt, rk[:, 0, ts(m, 128)], hT[c][:, 0, :],
                                start=False, stop=False,
                            )
                            nc.tensor.matmul(
                                pt, rk[:, 1, ts(m, 128)], hT[c][:, 1, :],
                                start=False, stop=True,
                            )
                    return pt2

                # deferred head state: (t, chunk) whose th-psum awaits drain
                def head_mm(c, t):
                    th = pschunk[c].tile([4, CW], F32, tag=f"psB{c}", name="th")
                    nc.tensor.matmul(
                        th[:, :], whead[:, 0, :], hT[c][:, 0, :],
                        start=True, stop=False,
                    )
                    nc.tensor.matmul(
                        th[:, :], whead[:, 1, :], hT[c][:, 1, :],
                        start=False, stop=True,
                    )
                    return th

                def head_drain(c, t, th):
                    stg = gsb.tile([4, CW], F32, tag=f"stg{c}", name="stg")
                    # +bias on all rows; relu only on lv rows via per-partition
                    # max operand (-inf on mean rows)
                    nc.vector.tensor_scalar(
                        out=stg, in0=th,
                        scalar1=hbias[:, 0:1], scalar2=hgate[:, 0:1],
                        op0=ALU.add, op1=ALU.max,
                    )
                    nc.sync.dma_start(out=oh_d[t, :, c, :], in_=stg)

                def chunk_step(c, t, xmm, rk, head_t=None):
                    """One full LSTM step for chunk c; PE flows i,f,g,o with
                    drains interleaved so no engine head-blocks >~1.2us."""
                    first = t == 0
                    if head_t is not None and head_t > 0:
                        # head for the PREVIOUS step of this chunk (h ready)
                        th = head_mm(c, head_t - 1)
                        head_drain(c, head_t - 1, th)
                    ti = mm_gate(c, t, 0, xmm, rk, "ti")
                    tf = mm_gate(c, t, 1, xmm, rk, "tf")
                    si = gsb.tile([128, 2, CW], BF, tag=f"si{c}", name="si")
                    nc.scalar.activation(out=si, in_=ti, func=AF.Sigmoid)
                    tgp = mm_gate(c, t, 2, xmm, rk, "tgp")
                    sf = gsb.tile([128, 2, CW], BF, tag=f"sf{c}", name="sf")
                    nc.scalar.activation(out=sf, in_=tf, func=AF.Sigmoid)
                    top = mm_gate(c, t, 3, xmm, rk, "top")
                    tg = gsb.tile([128, 2, CW], BF, tag=f"tg{c}", name="tg")
                    nc.scalar.activation(out=tg, in_=tgp, func=AF.Tanh)
                    if not first:
                        # fc on GPSIMD (off the DVE critical path)
                        nc.gpsimd.tensor_mul(fcs[c], sf, cT[c])
                    so = gsb.tile([128, 2, CW], BF, tag=f"so{c}", name="so")
                    nc.scalar.activation(out=so, in_=top, func=AF.Sigmoid)
                    tc_t = gsb.tile([128, 2, CW], BF, tag=f"tc{c}", name="tc")
                    if first:
                        nc.vector.tensor_mul(cT[c], si, tg)
                    else:
                        ig = gsb.tile([128, 2, CW], BF, tag=f"ig{c}", name="ig")
                        nc.vector.tensor_mul(ig, si, tg)  # bf16 2x
                        nc.vector.tensor_add(cT[c], fcs[c], ig)
                    nc.scalar.activation(out=tc_t, in_=cT[c], func=AF.Tanh)
                    hT[c] = gsb.tile([128, 2, CW], BF, tag=f"hT{c}", name="hT")
                    nc.vector.tensor_mul(hT[c], so, tc_t)  # bf16 2x
                    if head_t is not None and head_t == T_DEC - 1:
                        th = head_mm(c, head_t)
                        head_drain(c, head_t, th)

                # ---- P2: encoder (P1 packs interleaved ahead of use) ----
                for c in range(NCH):
                    p1_pack(c, 0)
                for t in range(T_ENC):
                    for c in range(NCH):
                        if 1 <= t <= 9:
                            p1_pack(c, 2 * t)

                        def enc_xmm(pt, m, t=t, c=c):
                            nc.tensor.matmul(
                                pt, enck[:, ts(m, 128)], xh[:, t, c, :],
                                start=True, stop=False if t > 0 else True,
                            )

                        chunk_step(c, t, enc_xmm, encrk)

                # ---- P3: zdx = dec_k^T enc_h + dec_b; decoder t=0 gates
                # are fused straight from the zdx PSUM tiles (no inject) ----
                zdx = encsb.tile([128, 8, NCH, CW], BF, tag="zdx")
                GFN = [AF.Sigmoid, AF.Sigmoid, AF.Tanh, AF.Sigmoid]
                GTAG = ["si", "sf", "tg", "so"]
                t0g = {}
                for g in range(4):
                    pz = {}
                    for c in range(NCH):
                        slot = "A" if g % 2 == 0 else "B"
                        pz[c] = pschunk[c].tile(
                            [128, 2, CW], F32, tag=f"ps{slot}{c}", name="pz"
                        )
                        for mi in range(2):
                            m = g * 2 + mi
                            nc.tensor.matmul(
                                pz[c][:, mi, :], deck[:, 0, ts(m, 128)], hT[c][:, 0, :],
                                start=True, stop=False,
                            )
                            nc.tensor.matmul(
                                pz[c][:, mi, :], deck[:, 1, ts(m, 128)], hT[c][:, 1, :],
                                start=False, stop=False,
                            )
                            nc.tensor.matmul(
                                pz[c][:, mi, :], deckb[:, ts(m, 128)], ones[:, :],
                                start=False, stop=True,
                            )
                    for c in range(NCH):
                        gt = gsb.tile([128, 2, CW], BF, tag=f"{GTAG[g]}{c}", name="t0g")
                        nc.scalar.activation(out=gt, in_=pz[c], func=GFN[g])
                        t0g[(g, c)] = gt
                        if g <= 1:
                            nc.vector.tensor_copy(zdx[:, ds(g * 2, 2), c, :], pz[c])
                        else:
                            nc.scalar.activation(
                                out=zdx[:, ds(g * 2, 2), c, :], in_=pz[c], func=AF.Copy
                            )
                # decoder t=0 cell update (c0 = i*g, h0 = o*tanh(c0))
                for c in range(NCH):
                    nc.vector.tensor_mul(cT[c], t0g[(0, c)], t0g[(2, c)])
                    tc0 = gsb.tile([128, 2, CW], BF, tag=f"tc{c}", name="tc0")
                    nc.scalar.activation(out=tc0, in_=cT[c], func=AF.Tanh)
                    hT[c] = gsb.tile([128, 2, CW], BF, tag=f"hT{c}", name="hT0")
                    nc.vector.tensor_mul(hT[c], t0g[(3, c)], tc0)

                # ---- P4: decoder t>=1 (in-scan head, deferred one step) ----
                for t in range(1, T_DEC):
                    for c in range(NCH):

                        def dec_xmm(pt, m, c=c):
                            nc.tensor.matmul(
                                pt, ident[:, :], zdx[:, m, c, :],
                                start=True, stop=False,
                            )

                        chunk_step(c, t, dec_xmm, decrk, head_t=t)

    nc.compile()
    return nc


def _marshal(x, w_in_k, w_in_b, enc_k, enc_rk, enc_b,
             dec_k, dec_rk, dec_b, mean_k, mean_b, lv_k, lv_b):
    f = np.float32
    bf = ml_dtypes.bfloat16
    x = np.asarray(x, f)
    enck_ext = np.concatenate([np.asarray(enc_k, f), np.asarray(enc_b, f)[None, :]], 0)
    shared = {
        "w_in_k": np.ascontiguousarray(
            np.tile(
                np.pad(np.asarray(w_in_k, bf), ((0, 24), (0, 0))), (4, 1)
            )
        ),
        "w_in_b64": np.ascontiguousarray(np.asarray(w_in_b, f)[:, None]),
        "enc_k_ext": np.ascontiguousarray(enck_ext.astype(bf)),
        "enc_rk": np.ascontiguousarray(
            np.asarray(enc_rk, bf).reshape(2, 128, 4 * H).transpose(1, 0, 2)
        ),
        "dec_k": np.ascontiguousarray(
            np.asarray(dec_k, bf).reshape(2, 128, 4 * H).transpose(1, 0, 2)
        ),
        "dec_b": np.ascontiguousarray(np.asarray(dec_b, bf)[None, :]),
        "dec_rk": np.ascontiguousarray(
            np.asarray(dec_rk, bf).reshape(2, 128, 4 * H).transpose(1, 0, 2)
        ),
        "w_head": np.ascontiguousarray(
            np.concatenate([np.asarray(mean_k, f), np.asarray(lv_k, f)], 1)
            .astype(bf)
            .reshape(2, 128, 4)
            .transpose(1, 0, 2)
        ),
        "head_bias": np.ascontiguousarray(
            np.concatenate([np.asarray(mean_b, f), np.asarray(lv_b, f)])[:, None]
        ),
        "head_gate": np.array([[-3e38], [-3e38], [0.0], [0.0]], f),
        "ident": np.eye(128, dtype=bf),
        "ones": np.ones((1, T_ENC * BC), bf),
    }
    in_maps = []
    for c in range(N_CORES):
        xs = x[c * BC : (c + 1) * BC]  # (BC, 20, 8)
        m = dict(shared)
        arr = xs.transpose(2, 1, 0).astype(bf).reshape(8, T_ENC, NCH, CW)
        xtw = np.zeros((128, 7, NCH, CW), bf)
        for t in range(T_ENC):
            xtw[(t % 3) * 32 : (t % 3) * 32 + 8, t // 3] = arr[:, t]
        m["xt"] = xtw
        in_maps.append(m)
    return in_maps


def _assemble(results):
    outs = []
    for c in range(N_CORES):
        oh = results[c]["out_head"].reshape(T_DEC, 4, BC)  # (t, r, b)
        outs.append(oh.transpose(2, 0, 1))  # (BC, 15, 4)
    return np.ascontiguousarray(np.concatenate(outs, 0))


def _run(trace=False, **inputs):
    global LAST_RESULTS
    if not _NC_CACHE:
        _NC_CACHE.append(_build_nc())
    nc = _NC_CACHE[0]
    in_maps = _marshal(**inputs)
    LAST_RESULTS = bass_utils.run_bass_kernel_spmd(
        nc, in_maps, core_ids=list(range(N_CORES)), trace=trace
    )
    return _assemble(LAST_RESULTS.results)


def kernel(**inputs):
    return _run(trace=False, **inputs)
